# revision 1
# baseline (speedup 1.0000x reference)
"""BFP activation quantization kernel for Trainium2 (8 NeuronCores).

Problem: x (64, 256, 56, 56) fp32. Channels grouped in blocks of 32; each
block shares the max frexp-exponent emax; mantissas truncated to
`mantissa_bits` bits relative to 2^emax:
    q = trunc(x / 2^(emax-mb)) * 2^(emax-mb)

Identities (all verified bit-exact on hardware):
  - emax = frexp_exponent(max_block |x|): one abs-max reduce per block.
  - Pw = 2^(emax-1) = bits(max|x|) & 0x7F800000 (bitcast); scale Pq = Pw *
    2^-(mb-1); invf = reciprocal(Pq) = 2^(mb-emax) is exact (power of two).
  - ya = |x| * invf is exact (power-of-two scaling), ya in [0, 2^mb).
  - The fp32->int16 converter rounds to nearest-even, so trunc is built as
        trunc(ya) = max(rne16(ya - 0.5), rne16(ya + 0.5) - 1)
    (ya +- 0.5 are exactly representable: same-binade grids), which is
    exact for every case including integer ya and half ties.
  - q = trunc * Pq exact; sign restored bitwise: q |= x & 0x80000000.

Layout (per core, n-shard of 8 images):
  tile = 1 image; SBUF partition p = (b<8, sigma<16), free = (c<32, s<196),
  s contiguous in DRAM (784 B runs). The block reduce is a strided free-dim
  reduce; per-(block, spatial) scale factors are dense (128, 196) so all
  scale math is cheap; apply passes use free-broadcast operands.

Engines: SP issues all DMAs; DVE does the main math; ACT (own SBUF port,
fully parallel) produces the two rounded int16 candidates. The DVE stream
is software-pipelined one stage (front of tile t+1 hides ACT latency of
tile t). Raw bass with explicit semaphores: this environment's compiler
accepts only ~1 sync wait per instruction, so every wait is a standalone
wait_ge.

Sharding: data-parallel on N across 8 cores, no cross-core communication.
"""

import numpy as np

N_CORES = 8
N, C, H, W = 64, 256, 56, 56
HW = H * W                   # 3136
N_PER_CORE = N // N_CORES    # 8
B = 8                        # channel blocks
SIG = 16                     # spatial chunks per image
C_IN = 32                    # channels per block
S = HW // SIG                # 196
P = B * SIG                  # 128 partitions
TILES = N_PER_CORE           # 8 (one image per tile)
DMAS = B                     # dma_starts per tile per direction
INC = 16 * DMAS              # load-sem increment per tile (128)

TRACE = False
LAST_RESULTS = None
_CACHE = {}


def _build(mbits: int):
    import concourse.bass as bass
    from concourse import mybir

    nc = bass.Bass()
    x_in = nc.declare_dram_parameter(
        "x", [N_PER_CORE, C, HW], mybir.dt.float32, isOutput=False
    )
    q_out = nc.declare_dram_parameter(
        "q", [N_PER_CORE, C, HW], mybir.dt.float32, isOutput=True
    )
    src = x_in[:].rearrange("n (b c) (g s) -> n b g c s", c=C_IN, s=S)
    dst = q_out[:].rearrange("n (b c) (g s) -> n b g c s", c=C_IN, s=S)

    i32, f32, i16, bf16 = (
        mybir.dt.int32, mybir.dt.float32, mybir.dt.int16, mybir.dt.bfloat16
    )
    Alu = mybir.AluOpType
    SIGN = -0x80000000  # int32 immediate for the sign bit

    from contextlib import ExitStack
    es = ExitStack()
    with es:
        sb = lambda nm, shape, dt: es.enter_context(nc.sbuf_tensor(nm, shape, dt))
        X0 = sb("X0", [P, C_IN, S], f32); X1 = sb("X1", [P, C_IN, S], f32)
        AX = sb("AXt", [P, C_IN, S], i32)
        YA0 = sb("YA0", [P, C_IN, S], f32); YA1 = sb("YA1", [P, C_IN, S], f32)
        T16 = sb("T16", [P, C_IN, S], i16); U16 = sb("U16", [P, C_IN, S], i16)
        TR = sb("TRt", [P, C_IN, S], i16); QF = sb("QFt", [P, C_IN, S], f32)
        M = sb("Mt", [P, S], f32); Pt = sb("Ptt", [P, S], i32)
        Pq = sb("Pqt", [P, S], f32); invf = sb("invf", [P, S], f32)
        SC0 = sb("SC0", [P, S], bf16); SC1 = sb("SC1", [P, S], bf16)
        BNEG = sb("BNEG", [P, 1], f32); BPOS = sb("BPOS", [P, 1], f32)
        load_sem = es.enter_context(nc.semaphore())
        store_sem = es.enter_context(nc.semaphore())
        dve_sem = es.enter_context(nc.semaphore())
        act_sem = es.enter_context(nc.semaphore())
        block = es.enter_context(nc.Block())
        X = [X0, X1]
        YA = [YA0, YA1]
        SC = [SC0, SC1]
        c2_done = {}     # dve counter after C2(t)
        gp_done = {}     # dve counter after G'(t)
        i_done = {}      # dve counter after I(t)
        kctr = {"k": 0}

        def bc(ap):
            return ap.unsqueeze(1).broadcast_to((P, C_IN, S))

        @block.vector
        def _(vector):
            k = 0

            def step(inst):
                nonlocal k
                inst.then_inc(dve_sem, 1)
                k += 1
                vector.wait_ge(dve_sem, k)

            step(vector.memset(BNEG[:], -0.5))
            step(vector.memset(BPOS[:], 0.5))

            def front(t):
                nonlocal k
                xb = X[t % 2]
                vector.wait_ge(load_sem, INC * (t + 1))
                step(vector.tensor_reduce(
                    out=M[:], in_=xb[:].rearrange("p c s -> p s c"),
                    axis=mybir.AxisListType.X, op=Alu.max,
                    apply_absolute_value=True,
                ))
                step(vector.tensor_scalar(
                    out=Pt[:], in0=M[:].bitcast(i32),
                    scalar1=0x7F800000, scalar2=None, op0=Alu.bitwise_and,
                ))
                step(vector.tensor_scalar(
                    out=Pq[:], in0=Pt[:].bitcast(f32),
                    scalar1=float(2.0 ** (-(mbits - 1))), scalar2=None,
                    op0=Alu.mult,
                ))
                step(vector.reciprocal(out=invf[:], in_=Pq[:]))
                step(vector.tensor_copy(SC[t % 2][:], Pq[:]))
                step(vector.tensor_scalar(
                    out=AX[:], in0=xb[:].bitcast(i32),
                    scalar1=0x7FFFFFFF, scalar2=None, op0=Alu.bitwise_and,
                ))
                step(vector.tensor_tensor(
                    out=YA[t % 2][:], in0=AX[:].bitcast(f32),
                    in1=bc(invf[:]), op=Alu.mult,
                ))
                c2_done[t] = k

            def back(t):
                nonlocal k
                vector.wait_ge(act_sem, 2 * (t + 1))
                # trunc(ya) = max(t16, u16 - 1)
                step(vector.scalar_tensor_tensor(
                    out=TR[:], in0=U16[:], scalar=1.0, in1=T16[:],
                    op0=Alu.subtract, op1=Alu.max,
                ))
                gp_done[t] = k
                step(vector.tensor_tensor(
                    out=QF[:], in0=TR[:], in1=bc(SC[t % 2][:]), op=Alu.mult,
                ))
                step(vector.tensor_scalar(
                    out=X[t % 2][:].bitcast(i32),
                    in0=X[t % 2][:].bitcast(i32),
                    scalar1=SIGN, scalar2=None, op0=Alu.bitwise_and,
                ))
                step(vector.tensor_tensor(
                    out=YA[t % 2][:].bitcast(i32),
                    in0=X[t % 2][:].bitcast(i32),
                    in1=QF[:].bitcast(i32), op=Alu.bitwise_or,
                ))
                i_done[t] = k

            front(0)
            for t in range(1, TILES):
                front(t)
                back(t - 1)
            back(TILES - 1)
            kctr["k"] = k

        @block.scalar
        def _(scalar):
            a = 0
            for t in range(TILES):
                # self-throttle (sem-race discipline), free at runtime
                if t:
                    scalar.wait_ge(act_sem, a)
                need = c2_done[t] if t == 0 else gp_done[t - 1]
                scalar.wait_ge(dve_sem, need)
                scalar.activation(
                    out=T16[:], in_=YA[t % 2][:],
                    func=mybir.ActivationFunctionType.Copy,
                    bias=-0.5, scale=1.0,
                ).then_inc(act_sem, 1)
                a += 1
                scalar.activation(
                    out=U16[:], in_=YA[t % 2][:],
                    func=mybir.ActivationFunctionType.Copy,
                    bias=0.5, scale=1.0,
                ).then_inc(act_sem, 1)
                a += 1

        def issue_loads(sync, t):
            xb = X[t % 2]
            for b in range(B):
                sync.dma_start(
                    out=xb[b * SIG:(b + 1) * SIG], in_=src[t, b]
                ).then_inc(load_sem, 16)

        @block.sync
        def _(sync):
            issue_loads(sync, 0)
            sync.wait_ge(load_sem, INC)
            issue_loads(sync, 1)
            for t in range(TILES):
                sync.wait_ge(dve_sem, i_done[t])
                yb = YA[t % 2]
                for b in range(B):
                    sync.dma_start(
                        out=dst[t, b], in_=yb[b * SIG:(b + 1) * SIG]
                    ).then_inc(store_sem, 16)
                if t + 2 < TILES:
                    sync.wait_ge(store_sem, INC * (t + 1))
                    sync.wait_ge(load_sem, INC * (t + 2))
                    issue_loads(sync, t + 2)

    return nc


def kernel(activations, mantissa_bits, blk, **_ignored):
    global LAST_RESULTS
    from concourse.bass_utils import run_bass_kernel_spmd

    mbits = int(mantissa_bits)
    assert int(blk) == C_IN, f"kernel hardcodes blk=32, got {blk}"
    x = np.ascontiguousarray(np.asarray(activations), dtype=np.float32)
    assert x.shape == (N, C, H, W), x.shape

    if mbits not in _CACHE:
        _CACHE[mbits] = _build(mbits)
    nc = _CACHE[mbits]

    shards = x.reshape(N_CORES, N_PER_CORE, C, HW)
    in_maps = [{"x": shards[i]} for i in range(N_CORES)]
    res = run_bass_kernel_spmd(nc, in_maps, list(range(N_CORES)), trace=TRACE)
    LAST_RESULTS = res
    out = np.stack([res.results[i]["q"] for i in range(N_CORES)], axis=0)
    return out.reshape(N, C, H, W)

